# revision 1
# baseline (speedup 1.0000x reference)
"""TRN2 Bass kernel v2 for nn_Network: 3-layer MLP (256-512-512-256) with
custom per-node activation a0*tanh(x)*sin(a1*x+a2) + a3*x + a4, then softmax.

Key optimizations over v1:
- Custom DVE op fuses the sin range reduction (affine + magic-round +
  subtract) into one Vector instruction reading PSUM directly.
- Mixed-precision matmuls: L1/L2 split-bf16 (hi+lo, 3 passes), L3 plain bf16
  -- 1024-wide moving operands; softmax feature sum via a ones-matmul
  broadcast in PSUM (replaces one gpsimd all-reduce).
- ACT-table steering: tanh/sin/identity served by silu_and_others, exp/ln by
  natural_log_exp_and_others => ~2 table loads per mega instead of ~18.
- t/s/q intermediates in bf16 (2x DVE path eligible, half SBUF).
- One input DMA + one output DMA per mega ([128,2048], 8KB/partition
  contiguous).
"""
import numpy as np

import concourse.bass as bass
import concourse.bass_isa as bass_isa
import concourse.mybir as mybir
import concourse.tile as tile
from concourse import bacc
from concourse.bass_utils import run_bass_kernel_spmd

F32 = mybir.dt.float32
BF16 = mybir.dt.bfloat16
ACTF = mybir.ActivationFunctionType
ALU = mybir.AluOpType

N_CORES = 8
B = 65536
B_LOC = B // N_CORES            # 8192 rows per core
NB = 1024                       # batch columns per f-tile op
N_MEGA = B_LOC // NB            # 8
MM_N = 512                      # moving width limit for fp32 matmul
LAYERS = [256, 512, 512, 256]
C_RND = float(1.5 * 2 ** 23)    # fp32 round-to-int magic constant
TWO_PI = float(2 * np.pi)

# per-layer matmul mode: "f32" | "split" (bf16 hi+lo 3-pass) | "bf16"
MM_MODE = {1: "f32", 2: "split", 3: "split"}
import os as _os
if _os.environ.get("NN_MM_MODE"):
    MM_MODE = dict(zip((1, 2, 3), _os.environ["NN_MM_MODE"].split(","))) 

_CACHE = {}


# ---- custom DVE op: v = u - round(u), u = in0*s0 + s1 ---------------------
def _register_range_reduce():
    from concourse.dve_ops import (
        OPS, CUSTOM_DVE_SPECS, _SUB_OPCODE_FOR_NAME, _CUSTOM_DVE_ROW_BASE,
        DveOp,
    )
    for op in OPS:
        if op.name == "SIN_RANGE_REDUCE":
            return op
    from concourse.dve_spec import Spec, Src0, C0, C1, C2

    def _ref(in0, in1, s0, s1, imm2):
        f = np.float32
        u = (in0.astype(f) * (f(s0) if np.isscalar(s0) else s0.astype(f)))
        u = (u + (f(s1) if np.isscalar(s1) else s1.astype(f))).astype(f)
        w = (u + f(imm2)).astype(f)
        k = (w - f(imm2)).astype(f)
        return (u - k).astype(f)

    u = Src0 * C0 + C1
    op = DveOp(
        "SIN_RANGE_REDUCE",
        Spec(body=u - ((u + C2) - C2), reference=_ref),
        subdim=False,
        uops_sha={"v3": "3d790cc1ec454799", "v4": "7b3df4a652590112"},
    )
    idx = len(OPS)
    OPS.append(op)
    CUSTOM_DVE_SPECS[op.name] = op.spec
    _SUB_OPCODE_FOR_NAME[op.name] = _CUSTOM_DVE_ROW_BASE + idx
    return op


RANGE_REDUCE = _register_range_reduce()


def _patch_act_tables():
    """Steer Bacc's greedy ACT-table-load pass: claim tanh/sin/identity only
    in silu_and_others and exp/ln only in natural_log_exp_and_others, so the
    layer chain runs switch-free.  Table list order/length preserved
    (act_func_set_id = act_info.json index)."""
    import concourse.bacc as bacc_mod
    if getattr(bacc_mod, "_nn_act_tables_patched", False):
        return
    _orig = bacc_mod.get_activation_tables
    KEEP_FULL = {"silu_and_others", "natural_log_exp_and_others"}
    STRIP = {ACTF.Tanh, ACTF.Sin, ACTF.Exp, ACTF.Ln, ACTF.Identity}

    def patched(arch):
        tabs = _orig(arch)
        return {name: (fs if name in KEEP_FULL else (set(fs) - STRIP))
                for name, fs in tabs.items()}

    bacc_mod.get_activation_tables = patched
    bacc_mod._nn_act_tables_patched = True


_patch_act_tables()


def _build_program(repeat=1, loop_n=None):
    key = ("prog", repeat, loop_n)
    if key in _CACHE:
        return _CACHE[key]

    n_pcols = 0
    pslots = {}
    for li, n_out in ((1, 512), (2, 512), (3, 256)):
        for f in range(n_out // 128):
            pslots[(li, f)] = n_pcols
            n_pcols += 4
    l3slots = {}
    for f in range(2):
        l3slots[f] = n_pcols
        n_pcols += 2

    nc = bacc.Bacc("TRN2", target_bir_lowering=False, debug=False,
                   num_devices=N_CORES)
    xt_d = nc.dram_tensor("xt", [128, 2 * B_LOC], F32, kind="ExternalInput").ap()
    on_d = nc.dram_tensor("on", [128, 128], F32, kind="ExternalInput").ap()
    pp_d = nc.dram_tensor("pp", [128, n_pcols], F32, kind="ExternalInput").ap()
    wdefs = []
    for li, (n_in, n_out) in enumerate(zip(LAYERS[:-1], LAYERS[1:]), start=1):
        ntiles = (n_in // 128) * (n_out // 128)
        mode = MM_MODE[li]
        if mode == "f32":
            wdefs.append((f"w{li}", [128, ntiles * 128], F32))
        elif mode == "bf16":
            wdefs.append((f"w{li}", [128, ntiles * 128], BF16))
        else:
            wdefs.append((f"w{li}h", [128, ntiles * 128], BF16))
            wdefs.append((f"w{li}l", [128, ntiles * 128], BF16))
    wd_aps = {nm: nc.dram_tensor(nm, shp, dt, kind="ExternalInput").ap()
              for nm, shp, dt in wdefs}
    yt_d = nc.dram_tensor("yt", [128, 2 * B_LOC], F32, kind="ExternalOutput").ap()

    with tile.TileContext(nc, num_cores=N_CORES) as tc:
        with tc.tile_pool(name="const", bufs=1) as cpool, \
             tc.tile_pool(name="io", bufs=1) as iopool, \
             tc.tile_pool(name="work", bufs=1) as wpool, \
             tc.tile_pool(name="psum", bufs=2, space="PSUM") as pspool:

            ones_t = cpool.tile([128, 128], F32)
            nc.sync.dma_start(ones_t[:], on_d)
            pp = cpool.tile([128, n_pcols], F32)
            nc.sync.dma_start(pp[:], pp_d)
            wtiles = {}
            for nm, shp, dt in wdefs:
                wtl = cpool.tile(shp, dt, name=f"wt_{nm}")
                nc.sync.dma_start(wtl[:], wd_aps[nm])
                wtiles[nm] = wtl

            def wslice(nm, li, k, f):
                nf = LAYERS[li] // 128
                o = (k * nf + f) * 128
                return wtiles[nm][:, o:o + 128]

            def pcol(li, f, idx):
                o = pslots[(li, f)] + idx
                return pp[:, o:o + 1]

            def l3col(f, idx):
                o = l3slots[f] + idx
                return pp[:, o:o + 1]

            import contextlib
            loop_ctx = tc.For_i(0, loop_n, 1) if loop_n else contextlib.nullcontext()
            with loop_ctx:
              for rep_m in range(N_MEGA * repeat):
                  m = rep_m % N_MEGA
                  csl = slice(m * 2048, (m + 1) * 2048)
                  xin = iopool.tile([128, 2048], F32, tag="xin", bufs=3,
                                    name=f"xin_{rep_m}")
                  nc.sync.dma_start(xin[:], xt_d[:, csl])
                  if MM_MODE[1] == "f32":
                      g_prev = [xin[:, 0:1024], xin[:, 1024:2048]]
                  else:
                      xh = iopool.tile([128, 2048], BF16, tag="xh", bufs=3,
                                       name=f"xh_{rep_m}")
                      nc.scalar.activation(xh[:], xin[:], ACTF.Identity)
                      xl = iopool.tile([128, 2048], BF16, tag="xl", bufs=3,
                                       name=f"xl_{rep_m}")
                      nc.vector.tensor_tensor(xl[:], xin[:], xh[:],
                                              ALU.subtract)
                      g_prev = ([xh[:, 0:1024], xh[:, 1024:2048]],
                                [xl[:, 0:1024], xl[:, 1024:2048]])

                  for li, (n_in, n_out) in enumerate(
                          zip(LAYERS[:-1], LAYERS[1:]), start=1):
                      nk, nf = n_in // 128, n_out // 128
                      mode = MM_MODE[li]
                      g_next = []
                      for f in range(nf):
                          ps = pspool.tile([128, NB], F32, tag="x", bufs=3,
                                           name=f"ps_{rep_m}_{li}_{f}")
                          if mode == "f32":
                              for k in range(nk):
                                  for b in range(NB // MM_N):
                                      bs = slice(b * MM_N, (b + 1) * MM_N)
                                      nc.tensor.matmul(
                                          ps[:, bs],
                                          wslice(f"w{li}", li, k, f),
                                          g_prev[k][:, bs],
                                          start=(k == 0), stop=(k == nk - 1))
                          elif mode == "bf16":
                              for k in range(nk):
                                  for b in range(NB // MM_N):
                                      bs = slice(b * MM_N, (b + 1) * MM_N)
                                      nc.tensor.matmul(
                                          ps[:, bs], wslice(f"w{li}", li, k, f),
                                          g_prev[k][:, bs],
                                          start=(k == 0), stop=(k == nk - 1))
                          else:  # split: Wh.gh + Wh.gl + Wl.gh
                              gh, gl = g_prev
                              # same stationary tile back-to-back where
                              # possible: per k, Wh serves gh then gl
                              steps = []
                              for k in range(nk):
                                  steps.append((f"w{li}h", k, gh[k]))
                                  steps.append((f"w{li}h", k, gl[k]))
                              for k in range(nk):
                                  steps.append((f"w{li}l", k, gh[k]))
                              n_mm = len(steps)
                              for b in range(NB // MM_N):
                                  bs = slice(b * MM_N, (b + 1) * MM_N)
                                  for i_mm, (wnm, k, gsrc) in enumerate(steps):
                                      nc.tensor.matmul(
                                          ps[:, bs], wslice(wnm, li, k, f),
                                          gsrc[:, bs],
                                          start=(i_mm == 0),
                                          stop=(i_mm == n_mm - 1))
                          # t = tanh(ps + c)   [ACT, bf16 out]
                          t_t = wpool.tile([128, NB], BF16, tag="t", bufs=3,
                                           name=f"t_{rep_m}_{li}_{f}")
                          nc.scalar.activation(t_t[:], ps[:], ACTF.Tanh,
                                               bias=pcol(li, f, 0), scale=1.0)
                          # v = u - round(u), u = ps*sinScale + sinBias [DVE]
                          v_t = wpool.tile([128, NB], F32, tag="v", bufs=3,
                                           name=f"v_{rep_m}_{li}_{f}")
                          nc.vector._custom_dve(
                              RANGE_REDUCE, out=v_t[:], in0=ps[:],
                              s0=pcol(li, f, 1), s1=pcol(li, f, 2),
                              imm2=C_RND)
                          # s = sin(2*pi*v)   [ACT, bf16 out]
                          s_t = wpool.tile([128, NB], BF16, tag="s", bufs=3,
                                           name=f"s_{rep_m}_{li}_{f}")
                          nc.scalar.activation(s_t[:], v_t[:], ACTF.Sin,
                                               bias=0.0, scale=TWO_PI)
                          # p = t*s   [DVE tt, all-bf16 -> 2x eligible]
                          q_t = wpool.tile([128, NB], BF16, tag="q", bufs=3,
                                           name=f"q_{rep_m}_{li}_{f}")
                          nc.vector.tensor_tensor(q_t[:], t_t[:], s_t[:],
                                                  ALU.mult)
                          # g = (ps + c) + q   [DVE]
                          nxt_mode = MM_MODE.get(li + 1)
                          if nxt_mode == "split":
                              g_f = wpool.tile([128, NB], F32,
                                               tag=f"g{li}f", bufs=2,
                                               name=f"gf_{rep_m}_{li}_{f}")
                              nc.vector.scalar_tensor_tensor(
                                  g_f[:], q_t[:], pcol(li, f, 3), ps[:],
                                  ALU.mult, ALU.add)
                              g_h = wpool.tile([128, NB], BF16,
                                               tag=f"g{li}h", bufs=4,
                                               name=f"gh_{rep_m}_{li}_{f}")
                              nc.scalar.activation(g_h[:], g_f[:],
                                                   ACTF.Identity)
                              g_l = wpool.tile([128, NB], BF16,
                                               tag=f"g{li}l", bufs=4,
                                               name=f"gl_{rep_m}_{li}_{f}")
                              nc.vector.tensor_tensor(g_l[:], g_f[:], g_h[:],
                                                      ALU.subtract)
                              g_next.append((g_h, g_l))
                          else:
                              gdt = BF16 if nxt_mode == "bf16" else F32
                              g_t = wpool.tile([128, NB], gdt, tag=f"g{li}",
                                               bufs=(6 if li < 3 else 3),
                                               name=f"g_{rep_m}_{li}_{f}")
                              nc.vector.scalar_tensor_tensor(
                                  g_t[:], q_t[:], pcol(li, f, 3), ps[:],
                                  ALU.mult, ALU.add)
                              g_next.append(g_t)
                      if g_next and isinstance(g_next[0], tuple):
                          g_prev = ([p[0] for p in g_next],
                                    [p[1] for p in g_next])
                      else:
                          g_prev = g_next

                  # ---- softmax over 256 features (2 g3 tiles) ----
                  zs = []
                  for f in range(2):
                      z_t = wpool.tile([128, NB], F32, tag="z", bufs=3,
                                       name=f"z_{rep_m}_{f}")
                      nc.scalar.activation(z_t[:], g_prev[f][:], ACTF.Identity,
                                           bias=l3col(f, 1), scale=l3col(f, 0))
                      zs.append(z_t)
                  vm = wpool.tile([128, NB], F32, tag="vm", bufs=2,
                                  name=f"vm_{rep_m}")
                  nc.vector.tensor_tensor(vm[:], zs[0][:], zs[1][:], ALU.max)
                  mb = wpool.tile([128, NB], F32, tag="mb", bufs=2,
                                  name=f"mb_{rep_m}")
                  nc.gpsimd.partition_all_reduce(mb[:], vm[:], channels=128,
                                                 reduce_op=bass_isa.ReduceOp.max)
                  es = []
                  for f in range(2):
                      nc.vector.tensor_tensor(zs[f][:], zs[f][:], mb[:],
                                              ALU.subtract)
                      nc.scalar.activation(zs[f][:], zs[f][:], ACTF.Exp)
                      es.append(zs[f])
                  sm = pspool.tile([128, NB], F32, tag="sm", bufs=1,
                                   name=f"sm_{rep_m}")
                  for b in range(NB // MM_N):
                      bsl2 = slice(b * MM_N, (b + 1) * MM_N)
                      nc.tensor.matmul(sm[:, bsl2], ones_t[:], es[0][:, bsl2],
                                       start=True, stop=False)
                      nc.tensor.matmul(sm[:, bsl2], ones_t[:], es[1][:, bsl2],
                                       start=False, stop=True)
                  sb = wpool.tile([128, NB], F32, tag="sb", bufs=2,
                                  name=f"sb_{rep_m}")
                  nc.scalar.activation(sb[:], sm[:], ACTF.Ln)
                  nc.scalar.activation(sb[:], sb[:], ACTF.Exp, scale=-1.0)
                  yout = iopool.tile([128, 2048], F32, tag="yout", bufs=2,
                                     name=f"yout_{rep_m}")
                  for f in range(2):
                      nc.vector.tensor_tensor(yout[:, f * NB:(f + 1) * NB],
                                              es[f][:], sb[:], ALU.mult)
                  nc.sync.dma_start(yt_d[:, csl], yout[:])

    nc.compile()
    _CACHE[key] = (nc, pslots, l3slots, n_pcols, wdefs)
    return _CACHE[key]


def _build_null_program():
    if "null" in _CACHE:
        return _CACHE["null"]
    nc, pslots, l3slots, n_pcols, wdefs = _build_program()
    nc2 = bacc.Bacc("TRN2", target_bir_lowering=False, debug=False,
                    num_devices=N_CORES)
    xt_d = nc2.dram_tensor("xt", [128, 2 * B_LOC], F32, kind="ExternalInput").ap()
    nc2.dram_tensor("on", [128, 128], F32, kind="ExternalInput")
    nc2.dram_tensor("pp", [128, n_pcols], F32, kind="ExternalInput")
    for nm, shp, dt in wdefs:
        nc2.dram_tensor(nm, shp, dt, kind="ExternalInput")
    yt_d = nc2.dram_tensor("yt", [128, 2 * B_LOC], F32, kind="ExternalOutput").ap()
    with tile.TileContext(nc2, num_cores=N_CORES) as tc:
        with tc.tile_pool(name="sb", bufs=1) as pool:
            t0 = pool.tile([128, 128], F32)
            nc2.sync.dma_start(t0[:], xt_d[0:128, 0:128])
            nc2.sync.dma_start(yt_d[0:128, 0:128], t0[:])
    nc2.compile()
    _CACHE["null"] = nc2
    return nc2


def _prep_host(inputs, repeat=1, loop_n=None):
    """Fold params on host (fp64) and pack device input tensors."""
    import jax.numpy as jnp

    W = [None, inputs["W1"].astype(np.float64), inputs["W2"].astype(np.float64),
         inputs["W3"].astype(np.float64)]
    bvec = [None, inputs["b1"].astype(np.float64), inputs["b2"].astype(np.float64),
            inputs["b3"].astype(np.float64)]
    a = [None, inputs["a1"].astype(np.float64), inputs["a2"].astype(np.float64),
         inputs["a3"].astype(np.float64)]

    a3c = [None] + [np.maximum(a[li][:, 3], 1e-20) for li in (1, 2, 3)]
    r = [None] + [a[li][:, 0] / a3c[li] for li in (1, 2, 3)]

    Wp = [None, W[1],
          W[2] * a3c[1][:, None],
          W[3] * a3c[2][:, None]]
    c = [None, bvec[1],
         W[2].T @ a[1][:, 4] + bvec[2],
         W[3].T @ a[2][:, 4] + bvec[3]]
    # effective bias with the per-layer +C passthrough folded forward
    # (device computes g' = g - C; X_l = ps_l + C_l stays exact)
    C = [None, c[1],
         None, None]
    C[2] = c[2] + W[2].T @ (a3c[1] * C[1])
    C[3] = c[3] + W[3].T @ (a3c[2] * C[2])

    nc, pslots, l3slots, n_pcols, wdefs = _build_program(repeat, loop_n)

    def tile_cat(Wm, li):
        n_in, n_out = LAYERS[li - 1], LAYERS[li]
        nf = n_out // 128
        return np.concatenate(
            [Wm[k * 128:(k + 1) * 128, f * 128:(f + 1) * 128]
             for k in range(n_in // 128) for f in range(nf)], axis=1)

    wmaps = {}
    for li in (1, 2, 3):
        Wf = tile_cat(Wp[li].astype(np.float32), li)
        mode = MM_MODE[li]
        if mode == "f32":
            wmaps[f"w{li}"] = Wf
        elif mode == "bf16":
            wmaps[f"w{li}"] = np.asarray(jnp.asarray(Wf, dtype=jnp.bfloat16))
        else:
            hi = np.asarray(jnp.asarray(Wf, dtype=jnp.bfloat16))
            lo = np.asarray(jnp.asarray(Wf - np.asarray(hi, np.float32),
                                        dtype=jnp.bfloat16))
            wmaps[f"w{li}h"] = hi
            wmaps[f"w{li}l"] = lo

    pp = np.zeros((128, n_pcols), np.float32)
    inv2pi = 1.0 / (2 * np.pi)
    for (li, f), o in pslots.items():
        sl = slice(f * 128, (f + 1) * 128)
        pp[:, o + 0] = C[li][sl].astype(np.float32)
        pp[:, o + 1] = (a[li][sl, 1] * inv2pi).astype(np.float32)
        pp[:, o + 2] = ((a[li][sl, 1] * C[li][sl] + a[li][sl, 2]) * inv2pi
                        ).astype(np.float32)
        pp[:, o + 3] = r[li][sl].astype(np.float32)
    for f, o in l3slots.items():
        sl = slice(f * 128, (f + 1) * 128)
        pp[:, o + 0] = a3c[3][sl].astype(np.float32)
        pp[:, o + 1] = (a3c[3][sl] * C[3][sl] + a[3][sl, 4]).astype(np.float32)

    ones = np.ones((128, 128), np.float32)
    data = inputs["data"].astype(np.float32)
    in_maps = []
    for i in range(N_CORES):
        dT = data[i * B_LOC:(i + 1) * B_LOC, :].T  # [256, B_LOC]
        xp = np.ascontiguousarray(
            dT.reshape(2, 128, N_MEGA, NB).transpose(1, 2, 0, 3)
            .reshape(128, 2 * B_LOC))
        in_maps.append({"xt": xp, "on": ones, "pp": pp, **wmaps})
    return nc, in_maps


def kernel(**inputs):
    nc, in_maps = _prep_host(inputs)
    res = run_bass_kernel_spmd(nc, in_maps, list(range(N_CORES)))
    out = np.empty((B, LAYERS[-1]), np.float32)
    for i in range(N_CORES):
        yp = res.results[i]["yt"]  # [128, 16384]
        Y = yp.reshape(128, N_MEGA, 2, NB).transpose(1, 3, 2, 0)
        out[i * B_LOC:(i + 1) * B_LOC, :] = Y.reshape(B_LOC, 256)
    return out


if __name__ == "__main__":
    rng = np.random.default_rng(0)
    inp = {"data": rng.standard_normal((B, 256), dtype=np.float32)}
    for i, (n_in, n_out) in enumerate(zip(LAYERS[:-1], LAYERS[1:])):
        inp[f"W{i+1}"] = rng.random((n_in, n_out), dtype=np.float32)
        inp[f"b{i+1}"] = np.zeros((n_out,), np.float32)
        inp[f"a{i+1}"] = rng.random((n_out, 5), dtype=np.float32)
    y = kernel(**inp)
    print("out", y.shape, y.dtype, y.min(), y.max())
    import numpy as _np
    def _act(x, aa):
        return (aa[:, 0] * _np.tanh(x) * _np.sin(aa[:, 1] * x + aa[:, 2])
                + aa[:, 3] * x + aa[:, 4])
    h = inp["data"].astype(_np.float64)
    for i in range(3):
        h = _act(h @ inp[f"W{i+1}"].astype(_np.float64),
                 inp[f"a{i+1}"].astype(_np.float64))
    e = _np.exp(h - h.max(1, keepdims=True))
    ref = e / e.sum(1, keepdims=True)
    rel = _np.linalg.norm(y - ref) / _np.linalg.norm(ref)
    print("rel err vs cpu ref:", rel)

